# revision 26
# baseline (speedup 1.0000x reference)
"""CPT attention (QKV+LoRA -> fake-quant KV -> causal attention -> proj+LoRA)
as a Bass/Tile kernel on 8 TRN2 NeuronCores.

Sharding: data parallel over batch (2) x tensor parallel over heads (16/4=4
per core), Megatron-style. Each core computes qkv for its 4 heads from the
full hidden_states[b], runs causal attention locally, and produces a partial
projection output [T, C]; the host sums the 4 tensor-parallel partials per
batch and adds b_proj.

Device-side design (v2 — k-major exp pipeline):
- All matmul operands f16 (1 col/cycle on PE), fp32 PSUM accum.
- Q for all four T-blocks is computed first; then per k-block tb the kernel
  emits K(tb), V(tb) and score chunks (hp, j, qb) for ALL q-blocks qb >= tb,
  so the exp stream on the Scalar engine starts ~10us in and runs
  continuously instead of bursting at the end of each q-block (the Scalar
  engine is a hard ~100us floor: causal scores / 128 partitions at ~1ns/elem).
- Scores are computed transposed (S^T[k, q]); softmax denominators come from
  a ones column appended to V (PV emits them as output column 65).
- kv_zp == 0 fast path: 1/kv_scale is folded into the K/V weight columns on
  the host so fake-quant is 2 dual-op DVE instructions (round-via-MAGIC and
  clip fused: max(x+MAGIC, MAGIC) then min(..., MAGIC+255) - MAGIC).
- exp() uses flat [128,1024] access patterns on off-diagonal chunks; diagonal
  chunks are masked with a 0/1 triangle multiply on the (otherwise idle)
  GpSimd engine.
- ex tiles are placed in manually managed SBUF slots (freed when PV(qb)
  retires) because their lifetimes don't fit a FIFO tile pool.
- Inputs are DMA'd with one wide strided descriptor per tensor chunk (the
  SP queue otherwise serializes ~50 issues x 0.7us at kernel start).
- LoRA / bias contributions are compiled out when the corresponding inputs
  are all-zero (separate cached program variants), as is the zp path.
"""

import numpy as np

import concourse.bass as bass
import concourse.bacc as bacc
import concourse.mybir as mybir
import concourse.tile as tile
from concourse.bass_utils import run_bass_kernel_spmd

AF = mybir.ActivationFunctionType
OP = mybir.AluOpType

B, T, C = 2, 2048, 1024
H, HD = 16, 64
R = 16
ALPHA_OVER_R = 2.0
QMAX = 255.0
MAGIC = 12582912.0  # 1.5 * 2**23: fp32 add/sub rounds to nearest-even integer
N_CORES = 8
HPC = 4  # heads per core
CH = HPC * HD  # 256 channels (per each of q/k/v) per core
NT = T // 128  # 16 T-tiles
NC_ = C // 128  # 8 C-tiles
F16 = mybir.dt.float16
F32 = mybir.dt.float32


def _build_body(nc, tc, d, use_bias, use_lora_attn, use_lora_proj, use_zp):
    import contextlib

    ctx = contextlib.ExitStack()
    with ctx:
        persist = ctx.enter_context(tc.tile_pool(name="persist", bufs=1))
        fqp = ctx.enter_context(tc.tile_pool(name="fqp", bufs=4))
        outp = ctx.enter_context(tc.tile_pool(name="outp", bufs=3))
        rcpp = ctx.enter_context(tc.tile_pool(name="rcpp", bufs=4))
        attnNp = ctx.enter_context(tc.tile_pool(name="attnNp", bufs=2))
        attnTp = ctx.enter_context(tc.tile_pool(name="attnTp", bufs=2))
        psS = ctx.enter_context(
            tc.tile_pool(name="psS", bufs=2, space=bass.MemorySpace.PSUM)
        )
        psB = ctx.enter_context(
            tc.tile_pool(name="psB", bufs=2, space=bass.MemorySpace.PSUM)
        )
        psV = ctx.enter_context(
            tc.tile_pool(name="psV", bufs=2, space=bass.MemorySpace.PSUM)
        )

        # ---- constants (DMA'd after the start-critical big tensors) ----
        consts = persist.tile([128, 4], F32, tag="consts", name="consts")
        inv_ap = consts[:, 0:1]
        zp_ap = consts[:, 1:2]
        es_ap = consts[:, 3:4]  # 0.125 * kv_scale (scores use integer-valued K)
        id16 = persist.tile([128, 128], F16, tag="id16", name="id16")
        maskt = persist.tile([128, 128], F16, tag="maskt", name="maskt")
        vcol = persist.tile([128, HPC], F16, tag="vcol", name="vcol")
        if use_bias:
            ones_row = persist.tile([1, 512], F16, tag="ones_row", name="ones_row")
            nc.gpsimd.memset(ones_row[:, :], 1.0)
            bqk_row = persist.tile([1, 2 * CH], F16, tag="bqk_row", name="bqk_row")
            nc.sync.dma_start(bqk_row[:, :], d["bqk"][:, :])
            bv_row = persist.tile([1, CH], F16, tag="bv_row", name="bv_row")
            nc.sync.dma_start(bv_row[:, :], d["bv"][:, :])

        # ---- persistent f16 tensors (DMA'd pre-transposed from host) ----
        # xT lives in one tile PER T-BLOCK (j-major inside) so each block is a
        # single DMA issue AND the first Q/K matmuls depend only on block 0's
        # DMA rather than on all of xT.
        xTblk = [
            persist.tile([128, NC_ * 512], F16, tag=f"xTb{tb}", name=f"xTb{tb}")
            for tb in range(4)
        ]

        def xs(j, c0, c1):
            """xT[j][:, c0:c1] — must stay within one 512-col T-block."""
            tb = c0 // 512
            assert c1 <= (tb + 1) * 512
            return xTblk[tb][:, j * 512 + (c0 - tb * 512) : j * 512 + (c1 - tb * 512)]
        # q and k weight halves in separate tiles: K(0) (the head of the exp
        # dependency chain) only waits for the 0.5MB k-half DMA.
        wqkQa = persist.tile([128, NC_ * CH], F16, tag="wqkQa", name="wqkQa")
        wqkKa = persist.tile([128, NC_ * CH], F16, tag="wqkKa", name="wqkKa")
        wvTa = persist.tile([128, NC_ * CH], F16, tag="wvTa", name="wvTa")
        wvT = [wvTa[:, j * CH : (j + 1) * CH] for j in range(NC_)]
        wpTa = persist.tile([128, 2 * C], F16, tag="wpTa", name="wpTa")
        wpT = [wpTa[:, i * C : (i + 1) * C] for i in range(2)]
        qkT = [
            persist.tile([128, T], F16, tag=f"qkT{i}", name=f"qkT{i}") for i in range(4)
        ]
        Vaug = [
            persist.tile([128, HPC * (HD + 1)], F16, tag=f"Vaug{t}", name=f"Vaug{t}")
            for t in range(NT)
        ]
        if use_lora_attn:
            AatTa = persist.tile([128, NC_ * R], F16, tag="AatTa", name="AatTa")
            AatT = [AatTa[:, j * R : (j + 1) * R] for j in range(NC_)]
            BqkT = persist.tile([R, 2 * CH], F16, tag="BqkT", name="BqkT")
            BvT = persist.tile([R, CH], F16, tag="BvT", name="BvT")
            LT = persist.tile([R, T], F16, tag="LT", name="LT")
        if use_lora_proj:
            ApT = [
                persist.tile([128, R], F16, tag=f"ApT{i}", name=f"ApT{i}")
                for i in range(2)
            ]
            BpT = persist.tile([R, C], F16, tag="BpT", name="BpT")
            LpT = persist.tile([R, T], F16, tag="LpT", name="LpT")

        # ---- single-issue wide DMAs, ordered for earliest compute start ----
        def dma_x_blk(tb, eng=None):
            dst = xTblk[tb][:, :].rearrange("p (j w) -> p j w", j=NC_)
            src = d["xT"].rearrange("(j p) t -> p j t", p=128)[
                :, :, tb * 512 : (tb + 1) * 512
            ]
            (eng or nc.sync).dma_start(dst, src)

        # split across two HWDGE queues (sync + scalar), the two start-critical
        # tensors (k-weights, x block 0) leading one queue each
        wqkv = d["wqkT"].rearrange("(j p) w -> p j w", p=128)
        nc.sync.dma_start(
            wqkKa[:, :].rearrange("p (j w) -> p j w", j=NC_), wqkv[:, :, CH : 2 * CH]
        )
        dma_x_blk(0, eng=nc.scalar)
        nc.sync.dma_start(
            wqkQa[:, :].rearrange("p (j w) -> p j w", j=NC_), wqkv[:, :, 0:CH]
        )
        nc.scalar.dma_start(consts[:, :], d["consts"][:, :])
        nc.scalar.dma_start(vcol[:, :], d["vinit"][:, :])
        nc.scalar.dma_start(maskt[:, :], d["masks"][:, :])
        dma_x_blk(1)
        nc.scalar.dma_start(
            wvTa[:, :].rearrange("p (j w) -> p j w", j=NC_),
            d["wvT"].rearrange("(j p) w -> p j w", p=128),
        )
        dma_x_blk(2, eng=nc.scalar)
        nc.sync.dma_start(
            wpTa[:, :].rearrange("p (i w) -> p i w", i=2),
            d["wpT"].rearrange("(i p) w -> p i w", p=128),
        )
        nc.sync.dma_start(id16[:, :], d["id16"][:, :])
        dma_x_blk(3)
        if use_lora_attn:
            nc.sync.dma_start(
                AatTa[:, :].rearrange("p (j w) -> p j w", j=NC_),
                d["aatT"].rearrange("(j p) w -> p j w", p=128),
            )
            nc.sync.dma_start(BqkT[:, :], d["bqkT"][:, :])
            nc.sync.dma_start(BvT[:, :], d["bvT"][:, :])
        if use_lora_proj:
            for i in range(2):
                nc.sync.dma_start(ApT[i][:, :], d["apT"][i * 128 : (i + 1) * 128, :])
            nc.sync.dma_start(BpT[:, :], d["bpT"][:, :])

        def fq_chain(dst_slice, src_ps, w, reshaped=False):
            """fake_quant. With use_zp=False the host folded 1/scale into the
            weights, so src is already x/scale and the chain is two dual-op
            instructions: round via +MAGIC (rne) fused with the lower clip,
            then upper clip fused with the -MAGIC unbias."""
            if not use_zp:
                t1 = fqp.tile([128, w], F32, tag="fq", name="fq1")
                nc.vector.tensor_scalar(t1[:, :], src_ps, MAGIC, MAGIC, OP.add, OP.max)
                src = (
                    t1[:, :].rearrange("p (h c) -> p h c", c=HD) if reshaped else t1[:, :]
                )
                nc.vector.tensor_scalar(
                    dst_slice, src, MAGIC + QMAX, MAGIC, OP.min, OP.subtract
                )
                return
            t1 = fqp.tile([128, w], F32, tag="fq", name="fq1")
            nc.vector.tensor_scalar(t1[:, :], src_ps, inv_ap, zp_ap, OP.mult, OP.add)
            t2 = fqp.tile([128, w], F32, tag="fq", name="fq2")
            nc.vector.tensor_scalar(t2[:, :], t1[:, :], 0.0, QMAX, OP.max, OP.min)
            t3 = fqp.tile([128, w], F32, tag="fq", name="fq3")
            nc.vector.tensor_scalar(t3[:, :], t2[:, :], MAGIC, MAGIC, OP.add, OP.subtract)
            src = t3[:, :].rearrange("p (h c) -> p h c", c=HD) if reshaped else t3[:, :]
            nc.vector.tensor_scalar(dst_slice, src, zp_ap, None, OP.subtract)

        # ---- emit helpers -------------------------------------------------
        def emit_lt(tb):
            ps = psB.tile([R, 512], F32, tag="mm", name="lt_ps")
            for j in range(NC_):
                nc.tensor.matmul(
                    ps[:, :],
                    AatT[j][:, :],
                    xs(j, tb * 512, (tb + 1) * 512),
                    start=(j == 0),
                    stop=(j == NC_ - 1),
                )
            nc.scalar.mul(LT[:, tb * 512 : (tb + 1) * 512], ps[:, :], ALPHA_OVER_R)

        def emit_qk_ct(tb, ct):
            """One 128-channel chunk (ct) of q or k for T-block tb. N=512."""
            ps = psB.tile([128, 512], F32, tag="mm", name="qk_ps")
            last = NC_ - 1 if not (use_lora_attn or use_bias) else None
            wa, cl = (wqkQa, ct) if ct < 2 else (wqkKa, ct - 2)
            for j in range(NC_):
                nc.tensor.matmul(
                    ps[:, :],
                    wa[:, j * CH + cl * 128 : j * CH + (cl + 1) * 128],
                    xs(j, tb * 512, (tb + 1) * 512),
                    start=(j == 0),
                    stop=(j == last),
                )
            if use_lora_attn:
                nc.tensor.matmul(
                    ps[:, :],
                    BqkT[:, ct * 128 : (ct + 1) * 128],
                    LT[:, tb * 512 : (tb + 1) * 512],
                    start=False,
                    stop=(not use_bias),
                )
            if use_bias:
                nc.tensor.matmul(
                    ps[:, :],
                    bqk_row[:, ct * 128 : (ct + 1) * 128],
                    ones_row[:, 0:512],
                    start=False,
                    stop=True,
                )
            dst = qkT[ct][:, tb * 512 : (tb + 1) * 512]
            if ct < 2:
                nc.vector.tensor_copy(dst, ps[:, :])
            else:
                fq_chain(dst, ps[:, :], 512)

        def emit_v_tile(t):
            nc.gpsimd.tensor_copy(
                Vaug[t][:, :].rearrange("p (h c) -> p h c", c=HD + 1)[:, :, HD],
                vcol[:, :],
            )
            ps = psB.tile([128, CH], F32, tag="mm", name="v_ps")
            last = NC_ - 1 if not (use_lora_attn or use_bias) else None
            for j in range(NC_):
                nc.tensor.matmul(
                    ps[:, :],
                    xs(j, t * 128, (t + 1) * 128),
                    wvT[j][:, :],
                    start=(j == 0),
                    stop=(j == last),
                )
            if use_lora_attn:
                nc.tensor.matmul(
                    ps[:, :],
                    LT[:, t * 128 : (t + 1) * 128],
                    BvT[:, :],
                    start=False,
                    stop=(not use_bias),
                )
            if use_bias:
                nc.tensor.matmul(
                    ps[:, :], ones_row[:, 0:128], bv_row[:, :], start=False, stop=True
                )
            vdst = Vaug[t][:, :].rearrange("p (h c) -> p h c", c=HD + 1)[:, :, 0:HD]
            fq_chain(vdst, ps[:, :], CH, reshaped=True)

        # ---- ex slot management (manual: lifetimes outlive a FIFO pool) ----
        ex_slots = []  # tile objects
        ex_free = []  # free slot indices
        ex_map = {}  # (hp, j, qb) -> slot idx

        def ex_alloc(hp, j, qb):
            if ex_free:
                s = ex_free.pop()
            else:
                s = len(ex_slots)
                ex_slots.append(
                    persist.tile([128, 1024], F16, tag=f"exs{s}", name=f"exs{s}")
                )
            ex_map[(hp, j, qb)] = s
            return ex_slots[s]

        def ex_get(hp, j, qb):
            return ex_slots[ex_map[(hp, j, qb)]]

        def ex_retire_qb(qb):
            for key in [k for k in ex_map if k[2] == qb]:
                ex_free.append(ex_map.pop(key))

        def emit_score_chunk(hp, j, qb):
            """S^T chunk for head pair hp, k-tile j, q-block qb (512 q cols,
            both heads side by side), followed by exp on the Scalar engine."""
            qt = qkT[hp]
            kt = qkT[2 + hp]
            jl = j - 4 * qb  # >=0 only possible when qb == j // 4 (diagonal)
            lo = max(jl, 0) * 128
            q0 = qb * 512
            ps = psS.tile([128, 1024], F32, tag="st", name="st_ps")
            nc.tensor.matmul(
                ps[:, lo:512],
                kt[0:64, j * 128 : (j + 1) * 128],
                qt[0:64, q0 + lo : q0 + 512],
                start=True,
                stop=True,
            )
            nc.tensor.matmul(
                ps[:, 512 + lo : 1024],
                kt[64:128, j * 128 : (j + 1) * 128],
                qt[64:128, q0 + lo : q0 + 512],
                start=True,
                stop=True,
            )
            ex = ex_alloc(hp, j, qb)
            if lo == 0:
                nc.scalar.activation(ex[:, :], ps[:, :], AF.Exp, scale=es_ap)
            else:
                exv = ex[:, :].rearrange("p (h q) -> p h q", q=512)[:, :, lo:512]
                psv = ps[:, :].rearrange("p (h q) -> p h q", q=512)[:, :, lo:512]
                nc.scalar.activation(exv, psv, AF.Exp, scale=es_ap)
            if jl >= 0:
                # diagonal k-tile: only q-slice qi == jl straddles the causal
                # boundary -> one triangle-mask multiply for both head halves
                exd = ex[:, :].rearrange("p (h q) -> p h q", q=512)[
                    :, :, jl * 128 : jl * 128 + 128
                ]
                nc.gpsimd.tensor_tensor(
                    exd,
                    exd,
                    maskt[:, :]
                    .rearrange("p (o f) -> p o f", o=1)
                    .broadcast_to([128, 2, 128]),
                    OP.mult,
                )

        def emit_pv(qb, between=None):
            """PV + normalize for q-block qb. Consumes ex(*, j<=4qb+3, qb).
            `between` optionally supplies thunks emitted before each of the 4
            (hp, hh) groups — tail exp work woven into the PV matmul stream."""
            attnN = attnNp.tile([128, 4 * CH], F16, tag="attnN", name=f"attnN{qb}")
            gi = 0
            for hp in range(2):
                for hh in range(2):
                    if between and gi < len(between) and between[gi] is not None:
                        between[gi]()
                    gi += 1
                    h = 2 * hp + hh
                    pvp4 = psV.tile([128, 4 * (HD + 1)], F32, tag="pv", name="pv_ps")
                    for qi in range(4):
                        qig = 4 * qb + qi
                        for j in range(qig + 1):
                            nc.tensor.matmul(
                                pvp4[:, qi * (HD + 1) : (qi + 1) * (HD + 1)],
                                ex_get(hp, j, qb)[
                                    :, hh * 512 + qi * 128 : hh * 512 + qi * 128 + 128
                                ],
                                Vaug[j][:, h * (HD + 1) : (h + 1) * (HD + 1)],
                                start=(j == 0),
                                stop=(j == qig),
                            )
                    pv4v = pvp4[:, :].rearrange("p (q c) -> p q c", c=HD + 1)
                    # denominator col of Vaug holds ~1/kv_scale so dequant
                    # scale folds into 1/sum (residual compensated in wpT/apT)
                    rcp4 = rcpp.tile([128, 4], F32, tag="rcp", name="rcp4")
                    nc.vector.reciprocal(rcp4[:, :], pv4v[:, :, HD])
                    dstv = attnN[:, :].rearrange("p (q c) -> p q c", c=CH)[
                        :, :, h * HD : (h + 1) * HD
                    ]
                    nc.vector.tensor_tensor(
                        dstv,
                        pv4v[:, :, 0:HD],
                        rcp4[:, :]
                        .rearrange("p (q o) -> p q o", o=1)
                        .broadcast_to([128, 4, HD]),
                        OP.mult,
                    )
            ex_retire_qb(qb)
            return attnN

        def emit_transpose(qb, attnN):
            attnT = attnTp.tile([128, 1024], F16, tag="attnT", name=f"attnT{qb}")
            # attnT layout: [128(ch within cb), cb*512 + qi*128 + qrow]
            for qp in range(2):  # pairs of q-tiles
                tp4 = psV.tile([128, 512], F16, tag="pv", name="tp4")
                for ti in range(2):
                    qi = 2 * qp + ti
                    for cb in range(2):
                        nc.tensor.transpose(
                            tp4[:, (2 * ti + cb) * 128 : (2 * ti + cb) * 128 + 128],
                            attnN[:, qi * CH + cb * 128 : qi * CH + (cb + 1) * 128],
                            id16[:, :],
                        )
                dstv = (
                    attnT[:, :]
                    .rearrange("p (cb q) -> p cb q", cb=2)[
                        :, :, qp * 256 : qp * 256 + 256
                    ]
                    .rearrange("p cb (ti f) -> p ti cb f", f=128)
                )
                srcv = tp4[:, :].rearrange("p (ti cb f) -> p ti cb f", ti=2, cb=2)
                nc.vector.tensor_copy(dstv, srcv)
            return attnT

        def emit_proj(qb, attnT):
            if use_lora_proj:
                emit_lp(qb, attnT)
            for qi in range(4):
                tt = 4 * qb + qi
                po_t = outp.tile([128, C], F16, tag="po", name=f"po{tt}")
                ps2s = [
                    psB.tile([128, 512], F32, tag="mm", name=f"pj_ps{nb}")
                    for nb in range(2)
                ]
                for cb in range(2):
                    for nb in range(2):
                        nc.tensor.matmul(
                            ps2s[nb][:, :],
                            attnT[:, cb * 512 + qi * 128 : cb * 512 + (qi + 1) * 128],
                            wpT[cb][:, nb * 512 : (nb + 1) * 512],
                            start=(cb == 0),
                            stop=(cb == 1 and not use_lora_proj),
                        )
                if use_lora_proj:
                    for nb in range(2):
                        nc.tensor.matmul(
                            ps2s[nb][:, :],
                            LpT[:, tt * 128 : (tt + 1) * 128],
                            BpT[:, nb * 512 : (nb + 1) * 512],
                            start=False,
                            stop=True,
                        )
                for nb in range(2):
                    nc.vector.tensor_copy(
                        po_t[:, nb * 512 : (nb + 1) * 512], ps2s[nb][:, :]
                    )
                nc.sync.dma_start(d["out"][tt * 128 : (tt + 1) * 128, :], po_t[:, :])

        def emit_lp(qb, attnT):
            ps = psB.tile([R, 512], F32, tag="mm", name="lp_ps")
            for cb in range(2):
                nc.tensor.matmul(
                    ps[:, :],
                    ApT[cb][:, :],
                    attnT[:, cb * 512 : (cb + 1) * 512],
                    start=(cb == 0),
                    stop=(cb == 1),
                )
            nc.scalar.mul(LpT[:, qb * 512 : (qb + 1) * 512], ps[:, :], ALPHA_OVER_R)

        # =============== emission schedule (k-major exp pipeline) ===========
        if use_lora_attn:
            for tb in range(4):
                emit_lt(tb)

        # K(0) first (head of the exp dependency chain), then Q(0); the
        # earliest exps are q-block 0's diagonal chunks (only block-0 deps).
        # q-block 0's cheap PV/proj fills the PE during the exp-heavy head;
        # off-diagonal chunks for later q-blocks are deferred so each
        # iteration's Scalar exp load roughly matches its PE work, and the
        # last diagonal exps are woven between PV(3)'s matmul groups.
        emit_qk_ct(0, 2)
        emit_qk_ct(0, 3)
        emit_qk_ct(0, 0)
        emit_qk_ct(0, 1)
        for hp in range(2):
            for j in range(4):
                emit_score_chunk(hp, j, 0)
        for t in range(4):
            emit_v_tile(t)
        # remaining Q blocks, each followed by its k-block-0 chunks
        for qb in range(1, 4):
            emit_qk_ct(qb, 0)
            emit_qk_ct(qb, 1)
            for j in range(4):
                for hp in range(2):
                    emit_score_chunk(hp, j, qb)
        attnN = emit_pv(0)
        attnT = emit_transpose(0, attnN)
        emit_proj(0, attnT)

        for tb in range(1, 3):
            emit_qk_ct(tb, 2)
            emit_qk_ct(tb, 3)
            for t in range(4 * tb, 4 * tb + 4):
                emit_v_tile(t)
            for hp in range(2):
                for j in range(4 * tb, 4 * tb + 4):
                    emit_score_chunk(hp, j, tb)
            attnN = emit_pv(tb)
            attnT = emit_transpose(tb, attnN)
            emit_proj(tb, attnT)
            # deferred off-diagonal chunks: exp work streaming while the PE
            # does the next block's K/V. Both use k-block 1 (tb=1 feeds qb=2,
            # tb=2 feeds qb=3); (qb=3, j in block 2) waits until tb=3.
            for j in range(4, 8):
                for hp in range(2):
                    emit_score_chunk(hp, j, tb + 1)

        # tb = 3
        emit_qk_ct(3, 2)
        emit_qk_ct(3, 3)
        for t in range(12, 16):
            emit_v_tile(t)
        for j in range(8, 12):  # (qb=3, j in block 2) held back until here
            for hp in range(2):
                emit_score_chunk(hp, j, 3)
        for j in range(12, 16):
            emit_score_chunk(0, j, 3)

        def _diag3(j0, j1):
            def f():
                for j in range(j0, j1):
                    emit_score_chunk(1, j, 3)
            return f

        attnN = emit_pv(3, between=[None, _diag3(12, 14), _diag3(14, 16), None])
        attnT = emit_transpose(3, attnN)
        emit_proj(3, attnT)


def _build_program(use_bias, use_lora_attn, use_lora_proj, use_zp):
    nc = bacc.Bacc("TRN2", target_bir_lowering=False, debug=False, num_devices=N_CORES)

    def din(name, shape, dt=F16):
        return nc.dram_tensor(name, shape, dt, kind="ExternalInput").ap()

    d = {
        "xT": din("xT", [C, T]),
        "wqkT": din("wqkT", [C, 2 * CH]),
        "wvT": din("wvT", [C, CH]),
        "wpT": din("wpT", [CH, C]),
        "aatT": din("aatT", [C, R]),
        "bqkT": din("bqkT", [R, 2 * CH]),
        "bvT": din("bvT", [R, CH]),
        "apT": din("apT", [CH, R]),
        "bpT": din("bpT", [R, C]),
        "bqk": din("bqk", [1, 2 * CH]),
        "bv": din("bv", [1, CH]),
        "consts": din("consts", [128, 4], F32),
        "id16": din("id16", [128, 128]),
        "masks": din("masks", [128, 128]),
        "vinit": din("vinit", [128, HPC]),
        "out": nc.dram_tensor("out", [T, C], F16, kind="ExternalOutput").ap(),
    }
    with tile.TileContext(nc) as tc:
        _build_body(nc, tc, d, use_bias, use_lora_attn, use_lora_proj, use_zp)
    nc.compile()
    _dedupe_ldweights(nc)
    return nc


def _dedupe_ldweights(nc):
    """Remove back-to-back InstLdweights that reload identical weights."""
    removed = 0
    pe = mybir.EngineType.PE
    for blk in nc.m.functions[0].blocks:
        insts = blk.instructions
        keep = []
        prev_key = None
        for inst in insts:
            if getattr(inst, "engine", None) != pe:
                keep.append(inst)
                continue
            t = type(inst).__name__
            if t == "InstLdweights":
                si = inst.sync_info
                clean = si is None or (not si.on_wait and not si.on_update)
                key = str(inst.ins[0])
                if clean and prev_key is not None and key == prev_key:
                    removed += 1
                    continue
                prev_key = key
            elif t == "InstMatmult":
                if getattr(inst, "is_transpose", False):
                    prev_key = None
            keep.append(inst)
        if len(keep) != len(insts):
            blk.instructions = keep
    return removed


_CACHE = {}


def get_program(use_bias=True, use_lora_attn=True, use_lora_proj=True, use_zp=True):
    key = (use_bias, use_lora_attn, use_lora_proj, use_zp)
    if key not in _CACHE:
        _CACHE[key] = _build_program(*key)
    return _CACHE[key]


def make_in_maps(
    hidden_states, W_attn, b_attn, A_attn, B_attn, W_proj, b_proj, A_proj, B_proj,
    kv_scale, kv_zp,
):
    f32, f16 = np.float32, np.float16
    hidden_states = np.asarray(hidden_states, f32)
    W_attn = np.asarray(W_attn, f32)
    b_attn = np.asarray(b_attn, f32)
    A_attn = np.asarray(A_attn, f32)
    B_attn = np.asarray(B_attn, f32)
    W_proj = np.asarray(W_proj, f32)
    A_proj = np.asarray(A_proj, f32)
    B_proj = np.asarray(B_proj, f32)
    scale = f32(np.asarray(kv_scale, f32).reshape(-1)[0])
    zp = f32(np.asarray(kv_zp, f32).reshape(-1)[0])
    use_zp = bool(zp != 0.0)

    consts = np.zeros((128, 4), f32)
    consts[:, 0] = f32(1.0) / scale
    consts[:, 1] = zp
    consts[:, 2] = scale
    consts[:, 3] = np.float32(0.125) * scale

    # id16 + causal masks
    id16 = np.eye(128, dtype=f16)
    iota_p = np.arange(128)[:, None]
    iota_f = np.arange(128)[None, :]
    masks = (iota_f - iota_p >= 0).astype(f16)  # [128,128] lower=0 triangle

    # ones-column value c ~= 1/scale (any finite c works: compensated exactly)
    c16 = np.float16(np.clip(1.0 / scale, 2.0 ** -14, 60000.0))
    corr = np.float64(scale) * np.float64(np.float32(c16))  # attnN = attn_true/corr
    vinit = np.full((128, HPC), c16, np.float16)

    # kv_zp == 0 fast path: fold 1/scale into K and V weight columns so the
    # PSUM already holds x/scale and fake-quant is a 2-instruction chain.
    kv_w_scale = np.float32(1.0) if use_zp else f32(1.0) / scale

    ct = lambda a: np.ascontiguousarray(a).astype(f16)
    xTs = [ct(hidden_states[b].T) for b in range(B)]
    bpT = ct(B_proj.T)

    in_maps = []
    for c in range(N_CORES):
        b = c // 4
        hg = c % 4
        qs = slice(hg * CH, (hg + 1) * CH)
        ks = slice(C + hg * CH, C + (hg + 1) * CH)
        vs = slice(2 * C + hg * CH, 2 * C + (hg + 1) * CH)
        wqk = np.concatenate([W_attn[qs], W_attn[ks] * kv_w_scale], axis=0)
        bqkl = np.concatenate([B_attn[qs], B_attn[ks] * kv_w_scale], axis=0)
        in_maps.append(
            {
                "xT": xTs[b],
                "wqkT": ct(wqk.T),
                "wvT": ct(W_attn[vs].T * kv_w_scale),
                "wpT": ct(W_proj[:, hg * CH : (hg + 1) * CH].T * corr),
                "aatT": ct(A_attn.T),
                "bqkT": ct(bqkl.T),
                "bvT": ct(B_attn[vs].T * kv_w_scale),
                "apT": ct(A_proj[:, hg * CH : (hg + 1) * CH].T * corr),
                "bpT": bpT,
                "bqk": ct(
                    np.concatenate([b_attn[qs], b_attn[ks] * kv_w_scale])[None, :]
                ),
                "bv": ct(b_attn[vs][None, :] * kv_w_scale),
                "consts": consts,
                "id16": id16,
                "masks": masks,
                "vinit": vinit,
            }
        )
    return in_maps


def variant_flags(b_attn, B_attn, B_proj, kv_zp=None):
    flags = (
        bool(np.any(np.asarray(b_attn))),
        bool(np.any(np.asarray(B_attn))),
        bool(np.any(np.asarray(B_proj))),
    )
    if kv_zp is None:
        return flags
    return flags + (bool(np.asarray(kv_zp).reshape(-1)[0] != 0.0),)


def assemble_output(results, b_proj):
    out = np.zeros((B, T, C), np.float32)
    for c in range(N_CORES):
        out[c // 4] += results[c]["out"].astype(np.float32)
    out += np.asarray(b_proj, np.float32)[None, None, :]
    return out


def kernel(**inputs):
    flags = variant_flags(
        inputs["b_attn"], inputs["B_attn"], inputs["B_proj"], inputs["kv_zp"]
    )
    nc = get_program(*flags)
    in_maps = make_in_maps(**inputs)
    res = run_bass_kernel_spmd(nc, in_maps, core_ids=list(range(N_CORES)))
    return assemble_output(res.results, inputs["b_proj"])
